# revision 44
# baseline (speedup 1.0000x reference)
"""Multi-head self-attention (QK^T -> softmax -> ctx -> linear) on 8 TRN2 cores.

Sharding: each core owns one (batch, query-block) shard: batch = core//4,
queries [qlo, qlo+512) with qlo = (core%4)*512. Attention needs all keys of
the core's batch, so keys are replicated per batch; no collectives needed.

All matmuls run in bf16 (fp32 PSUM accumulate). Per core (head h, its 512
queries q in 4 blocks of 128, all 2048 keys k in 16 blocks of 128):

  S_T[k, q]  = sum_d x[k, hd] * x[q, hd]          (PE; stationary xt, moving xq)
  P_T[k, q]  = exp(0.125 * S_T[k, q])             (ACT exp, or Pool pow for
                7 of 16 kb steps per pair: DVE stages the PSUM->SBUF copy and
                Pool computes (e^0.125)^s -- exp is the throughput wall, so
                it is spread over three engines)
  ctx[q, m]  = sum_k P_T[k, q] * xa[k, m]         (PE; stationary P_T 128q-slice,
                moving xa=[x|1|0] so col 64 is the softmax denominator; 66-wide
                output halves the PE column count vs the xa-stationary form)
  ctxn[q, i] = ctx[q, d] * (1/ctx[q, 64])         (DVE strided recip + per-
                partition tensor_scalar mul)
  ctxT       = DMA-transpose(ctxn)                (idle DMA xbar, no PE cost)
  out[q, o]  = sum_i ctxT[i, q] * W[o, i] + b[o]  (PE; bias via K=1 matmul)

Scheduling notes (TimelineSim cost model): Pool-offloaded steps give their
ctx matmuls 14 steps of pipeline slack (coarse sync, count-ordered with the
ACT stream) -- tighter couplings consistently lost ~10us to scheduler/cost-
model friction. ctx PSUM accumulation is bank-granular: one start=True
pending-zeroes a head's whole bank; the 4 query-block regions then rely on
byte-granular pending-zero, so accumulation order across kb is free.
Projection chunks 0..6 pre-accumulate into freed PSUM banks during the last
pair's normalization. ~148us vs 167us baseline; ACT ~98us, PE ~102us busy.
"""

import sys

for _p in ("/opt/trn_rl_repo", "/root/.axon_site/_ro/trn_rl_repo"):
    if _p not in sys.path:
        sys.path.append(_p)

import numpy as np
import ml_dtypes

import concourse.bacc as bacc
import concourse.mybir as mybir
import concourse.tile as tile

F32 = mybir.dt.float32
BF16 = mybir.dt.bfloat16

B, L, H, NH, DH = 2, 2048, 1024, 16, 64
NCORES = 8
QB = 512


def build_nc(L=2048, NH=16, DH=64, QB=512, H=1024):
    """One SPMD Bass program; per-core data differences live in the inputs."""
    KBLKS = L // 128            # key blocks of 128
    NPAIR = NH // 2             # head pairs (2 heads share a 128-part tile)
    AUG = DH + 2                # x + ones column + zero pad (ISA needs even width)
    HC = H // 128               # hidden chunks for the final matmul
    QSUB = QB // 128            # query sub-blocks of 128
    NSTEP = NPAIR * KBLKS
    SKEW = 6                    # ACT-region ctx matmuls trail scores by SKEW
    SKEWP = 13                  # Pool-region ctx matmuls trail further still
    # Every step's exp is split: ACT takes score columns [0:CA], Pool takes
    # [CA:2QB] via a DVE-staged PSUM->SBUF copy + pow. CA=640 aligns with the
    # 128-wide ctx stationary slices so each ctx matmul depends on only one
    # exp producer.
    CA = int(__import__('os').environ.get('CA', '1024'))
    # ctx_b's bank-zeroing start matmul lives in the ACT-region group, so the
    # ACT region must cover head-b qs0.
    assert CA >= QB + 128 or CA == 2 * QB
    BASE = float(np.exp(0.125))

    nc = bacc.Bacc("TRN2")
    xt = nc.declare_dram_parameter("xt", [NPAIR, 128, L], BF16, isOutput=False)
    xq = nc.declare_dram_parameter("xq", [NPAIR, 128, QB], BF16, isOutput=False)
    xa = nc.declare_dram_parameter("xa", [NH, 128, KBLKS * AUG], BF16, isOutput=False)
    wt = nc.declare_dram_parameter("wt", [128, HC * H], BF16, isOutput=False)
    biasrow = nc.declare_dram_parameter("biasrow", [1, H], BF16, isOutput=False)
    ones = nc.declare_dram_parameter("ones", [1, 128], BF16, isOutput=False)
    out = nc.declare_dram_parameter("out", [QB, H], F32, isOutput=True)

    with tile.TileContext(nc) as tc:
        with (
            tc.tile_pool(name="xt", bufs=3) as xt_pool,
            tc.tile_pool(name="xq", bufs=3) as xq_pool,
            tc.tile_pool(name="xa", bufs=6) as xa_pool,
            tc.tile_pool(name="p", bufs=11) as p_pool,
            tc.tile_pool(name="pp", bufs=10) as pp_pool,
            tc.tile_pool(name="scp", bufs=5) as scp_pool,
            tc.tile_pool(name="rc", bufs=4) as rc_pool,
            tc.tile_pool(name="ctxn", bufs=32) as ctxn_pool,
            tc.tile_pool(name="consts", bufs=1) as consts,
            tc.tile_pool(name="osb", bufs=2) as o_pool,
            tc.tile_pool(name="spsum", bufs=2, space="PSUM") as s_psum,
            tc.tile_pool(name="cpsum", bufs=4, space="PSUM") as c_psum,
        ):
            ones_t = consts.tile([1, 128], BF16)
            bias_t = consts.tile([1, H], BF16)
            base_t = consts.tile([128, 2 * QB], F32)
            nc.vector.memset(base_t[:], BASE)
            warm = consts.tile([128, 2], BF16)
            nc.scalar.activation(
                warm[:], base_t[:, 0:2], mybir.ActivationFunctionType.Exp
            )
            wt_t = consts.tile([128, HC * H], BF16)
            ctxT = [
                consts.tile([128, QB], BF16, tag=f"ctxT{c}", name=f"ctxT{c}")
                for c in range(HC)
            ]

            tiles = {}

            def emit_pair_dma(pr):
                xt_t = xt_pool.tile([128, L], BF16)
                xq_t = xq_pool.tile([128, QB], BF16)
                nc.sync.dma_start(xq_t[:], xq[pr])
                nc.sync.dma_start(xt_t[:, 0 : L // 4], xt[pr][:, 0 : L // 4])
                nc.sync.dma_start(xt_t[:, L // 4 : L // 2], xt[pr][:, L // 4 : L // 2])
                nc.sync.dma_start(xt_t[:, L // 2 :], xt[pr][:, L // 2 :])
                xa_a = xa_pool.tile([128, KBLKS * AUG], BF16)
                nc.sync.dma_start(xa_a[:], xa[2 * pr])
                xa_b = xa_pool.tile([128, KBLKS * AUG], BF16)
                nc.sync.dma_start(xa_b[:], xa[2 * pr + 1])
                # Full 2KB PSUM bank per head: start=True pending-zeroes the
                # whole bank, so the 4 qb accumulation regions share one
                # start (first matmul) and rely on byte-granular pending-zero.
                ctx_a = c_psum.tile([128, 512], F32, tag="c")
                ctx_b = c_psum.tile([128, 512], F32, tag="c")
                tiles[pr] = (xt_t, xq_t, xa_a, xa_b, ctx_a, ctx_b)

            def emit_norm_recips(pr):
                _, _, _, _, ctx_a, ctx_b = tiles[pr]
                rc_ab = rc_pool.tile([128, 2 * QSUB], F32)
                with nc.allow_low_precision(reason="softmax denom reciprocal"):
                    nc.vector.reciprocal(
                        rc_ab[:, 0:QSUB], ctx_a[:, DH : QSUB * AUG : AUG]
                    )
                    nc.vector.reciprocal(
                        rc_ab[:, QSUB : 2 * QSUB], ctx_b[:, DH : QSUB * AUG : AUG]
                    )
                return rc_ab

            ctxn_cur = [None]

            def emit_norm_mul(pr, qs, h, rc_ab):
                _, _, _, _, ctx_a, ctx_b = tiles[pr]
                asl = slice(qs * AUG, qs * AUG + DH)
                if h == 0:
                    ctxn_cur[0] = ctxn_pool.tile([128, 128], BF16, name="ctxn")
                    nc.vector.tensor_scalar(
                        ctxn_cur[0][:, 0:DH], ctx_a[:, asl],
                        rc_ab[:, qs : qs + 1], None,
                        op0=mybir.AluOpType.mult,
                    )
                else:
                    nc.vector.tensor_scalar(
                        ctxn_cur[0][:, DH:128], ctx_b[:, asl],
                        rc_ab[:, QSUB + qs : QSUB + qs + 1], None,
                        op0=mybir.AluOpType.mult,
                    )
                    nc.sync.dma_start(
                        ctxT[pr][:, qs * 128 : (qs + 1) * 128], ctxn_cur[0][:],
                        transpose=True,
                    )

            # Software-pipelined (pair, kb) stream. Steps kb in POOL_KB are
            # offloaded whole to the Pool engine (DVE stages the PSUM->SBUF
            # copy, Pool computes (e^0.125)^s); their ctx matmuls run at the
            # END of the same pair (kb 14/15), giving the slow pool chain
            # ~13 steps of slack with coarse-grained sync. The other steps
            # run ACT exp with ctx trailing by SKEW steps.
            POOL_KB = (0, 2, 4, 7, 9, 11, 13)
            emit_pair_dma(0)
            emit_pair_dma(1)
            pending = []
            pending_p = []
            norm_q = []
            pair_rc = {}
            act_ctx_done = set()

            ctx_cnt = {}

            def emit_ctx(prp, kbp, pp):
                cnt = ctx_cnt.get(prp, 0)
                ctx_cnt[prp] = cnt + 1
                first = cnt == 0
                last = cnt == KBLKS - 1
                _, _, xa_a, xa_b, ctx_a, ctx_b = tiles[prp]
                asl = slice(kbp * AUG, (kbp + 1) * AUG)
                for qs in range(QSUB):
                    qsl = slice(qs * 128, (qs + 1) * 128)
                    osl = slice(qs * AUG, (qs + 1) * AUG)
                    nc.tensor.matmul(
                        ctx_a[:, osl], pp[:, qsl], xa_a[:, asl],
                        start=(first and qs == 0), stop=last,
                        skip_group_check=True,
                    )
                for qs in range(QSUB):
                    qsl = slice(QB + qs * 128, QB + (qs + 1) * 128)
                    osl = slice(qs * AUG, (qs + 1) * AUG)
                    nc.tensor.matmul(
                        ctx_b[:, osl], pp[:, qsl], xa_b[:, asl],
                        start=(first and qs == 0), stop=last,
                        skip_group_check=True,
                    )
                if last:
                    norm_q.append(("r", prp))
                    for q_ in range(QSUB):
                        norm_q.append(("m", prp, q_))

            for gs in range(NSTEP + SKEW + 5 + KBLKS):
                if gs < NSTEP:
                    pr, kb = divmod(gs, KBLKS)
                    if kb == 0 and pr + 2 <= NPAIR - 1:
                        emit_pair_dma(pr + 2)
                    if pr == NPAIR - 2 and kb == 0:
                        nc.sync.dma_start(wt_t[:], wt[:])
                        nc.sync.dma_start(ones_t[:], ones[:])
                        nc.sync.dma_start(bias_t[:], biasrow[:])
                    xt_t, xq_t, xa_a, xa_b, _, _ = tiles[pr]
                    s_ab = s_psum.tile([128, 2 * QB], F32, tag="s")
                    ksl = slice(kb * 128, (kb + 1) * 128)
                    nc.tensor.matmul(
                        s_ab[:, 0:QB], xt_t[0:DH, ksl], xq_t[0:DH, :],
                        start=True, stop=True,
                    )
                    nc.tensor.matmul(
                        s_ab[:, QB : 2 * QB], xt_t[DH:128, ksl], xq_t[DH:128, :],
                        start=True, stop=True,
                    )
                    if kb in POOL_KB:
                        s_sb = scp_pool.tile([128, 2 * QB], F32)
                        nc.vector.tensor_copy(s_sb[:], s_ab[:])
                        p_pl = pp_pool.tile([128, 2 * QB], BF16, tag="pp")
                        nc.gpsimd.tensor_tensor(
                            p_pl[:], base_t[:], s_sb[:], op=mybir.AluOpType.pow
                        )
                        pending_p.append((pr, kb, p_pl))
                    else:
                        p_act = p_pool.tile([128, 2 * QB], BF16, tag="p")
                        nc.scalar.activation(
                            p_act[:], s_ab[:],
                            mybir.ActivationFunctionType.Exp, scale=0.125,
                        )
                        pending.append((pr, kb, p_act))
                else:
                    pr, kb = divmod(gs, KBLKS)
                    if not (pending or pending_p or norm_q):
                        break
                if len(pending) > SKEW or (gs >= NSTEP and pending):
                    prp, kbp, pp = pending.pop(0)
                    emit_ctx(prp, kbp, pp)
                if pending_p and (gs >= NSTEP or gs >= pending_p[0][0] * KBLKS + pending_p[0][1] + 10):
                    prp, kbp, pp = pending_p.pop(0)
                    emit_ctx(prp, kbp, pp)
                if norm_q and (gs >= NSTEP or (gs % KBLKS) in (6, 9, 10, 11, 12)):
                    op = norm_q.pop(0)
                    if op[0] == "r":
                        pair_rc[op[1]] = emit_norm_recips(op[1])
                    else:
                        emit_norm_mul(op[1], op[2], 0, pair_rc[op[1]])
                        emit_norm_mul(op[1], op[2], 1, pair_rc[op[1]])

            # Early projection: chunks 0..HC-2 for the first two query
            # blocks can accumulate while the last pair's norm finishes
            # (their ctxT chunks are complete; PSUM slots free as the last
            # exps retire).
            early_ps = {}
            for qs in (0, 1):
                qsl = slice(qs * 128, (qs + 1) * 128)
                out_ps = s_psum.tile([128, 2 * QB], F32, tag="s")
                early_ps[qs] = [out_ps[:, 0:QB], out_ps[:, QB : 2 * QB]]
                for c in range(HC - 1):
                    for ob in range(2):
                        nc.tensor.matmul(
                            early_ps[qs][ob], ctxT[c][:, qsl],
                            wt_t[:, c * H + ob * QB : c * H + (ob + 1) * QB],
                            start=(c == 0), stop=False,
                        )
            for qs in (2, 3):
                qsl = slice(qs * 128, (qs + 1) * 128)
                ps0 = c_psum.tile([128, 512], F32, tag="c")
                ps1 = c_psum.tile([128, 512], F32, tag="c")
                early_ps[qs] = [ps0[:, 0:QB], ps1[:, 0:QB]]
                for c in range(HC - 1):
                    for ob in range(2):
                        nc.tensor.matmul(
                            early_ps[qs][ob], ctxT[c][:, qsl],
                            wt_t[:, c * H + ob * QB : c * H + (ob + 1) * QB],
                            start=(c == 0), stop=False,
                        )
            for op in norm_q:
                if op[0] == "r":
                    pair_rc[op[1]] = emit_norm_recips(op[1])
                else:
                    emit_norm_mul(op[1], op[2], 0, pair_rc[op[1]])
                    emit_norm_mul(op[1], op[2], 1, pair_rc[op[1]])

            # Output projection: out[q, :] = sum_c ctxT_c[:, q].T @ wt_c + b
            # (moving operand is capped at 512 elements per matmul -> 2 blocks)
            c = HC - 1
            for qs in range(QSUB):
                qsl = slice(qs * 128, (qs + 1) * 128)
                for ob in range(2):
                    nc.tensor.matmul(
                        early_ps[qs][ob], ctxT[c][:, qsl],
                        wt_t[:, c * H + ob * QB : c * H + (ob + 1) * QB],
                        start=False, stop=False,
                    )
                    nc.tensor.matmul(
                        early_ps[qs][ob], ones_t[0:1, 0:128],
                        bias_t[0:1, ob * QB : (ob + 1) * QB],
                        start=False, stop=True,
                    )
            for qs in range(QSUB):
                o_sb = o_pool.tile([128, H], F32)
                for ob in range(2):
                    osl = slice(ob * QB, (ob + 1) * QB)
                    nc.vector.tensor_copy(o_sb[:, osl], early_ps[qs][ob])
                    nc.sync.dma_start(
                        out[qs * 128 : (qs + 1) * 128, ob * QB : (ob + 1) * QB],
                        o_sb[:, osl],
                    )
    nc.compile()
    return nc


def shard_inputs(key, W_ctx, b_ctx, L=2048, NH=16, DH=64, QB=512, H=1024):
    """Host-side prep of per-core input dicts (bf16)."""
    KBLKS = L // 128
    NPAIR = NH // 2
    AUG = DH + 2
    HC = H // 128
    Bv = key.shape[0]
    cores_per_batch = NCORES // Bv
    bf16 = ml_dtypes.bfloat16

    key = np.asarray(key, dtype=np.float32)
    xh = key.reshape(Bv, L, NH, DH)
    # xt: [B, NPAIR, 128, L], pair p rows 0:64 = head 2p, 64:128 = head 2p+1
    xt_full = np.ascontiguousarray(
        xh.transpose(0, 2, 3, 1).reshape(Bv, NPAIR, 2 * DH, L).astype(bf16)
    )
    # xa: [B, NH, 128, KBLKS*AUG] with ones in column kb*AUG+DH
    xa_full = np.empty((Bv, NH, 128, KBLKS * AUG), dtype=bf16)
    xa_view = xa_full.reshape(Bv, NH, 128, KBLKS, AUG)
    xa_view[..., DH] = 1.0
    xa_view[..., DH + 1] = 0.0
    xa_view[..., 0:DH] = (
        xh.reshape(Bv, KBLKS, 128, NH, DH).transpose(0, 3, 2, 1, 4).astype(bf16)
    )
    wt_host = np.ascontiguousarray(
        np.asarray(W_ctx, np.float32).T.reshape(HC, 128, H).transpose(1, 0, 2)
        .reshape(128, HC * H).astype(bf16)
    )
    bias_host = np.ascontiguousarray(
        np.asarray(b_ctx, np.float32).reshape(1, H).astype(bf16)
    )
    ones_host = np.ones((1, 128), dtype=bf16)

    in_maps = []
    meta = []
    for c in range(NCORES):
        b = c // cores_per_batch
        qlo = (c % cores_per_batch) * QB
        in_maps.append(
            {
                "xt": xt_full[b],
                "xq": np.ascontiguousarray(xt_full[b][:, :, qlo : qlo + QB]),
                "xa": xa_full[b],
                "wt": wt_host,
                "biasrow": bias_host,
                "ones": ones_host,
            }
        )
        meta.append((b, qlo))
    return in_maps, meta


_NC_CACHE = {}


def kernel(key, W_ctx, b_ctx):
    from concourse.bass_utils import run_bass_kernel_spmd

    key = np.asarray(key, dtype=np.float32)
    if "nc" not in _NC_CACHE:
        _NC_CACHE["nc"] = build_nc(L=L, NH=NH, DH=DH, QB=QB, H=H)
    nc = _NC_CACHE["nc"]
    in_maps, meta = shard_inputs(key, W_ctx, b_ctx, L=L, NH=NH, DH=DH, QB=QB, H=H)
    res = run_bass_kernel_spmd(nc, in_maps, list(range(NCORES)))
    outf = np.empty((B, L, H), dtype=np.float32)
    for c, (b, qlo) in enumerate(meta):
        outf[b, qlo : qlo + QB] = res.results[c]["out"]
    return outf


# revision 45
# speedup vs baseline: 1.0479x; 1.0479x over previous
"""Multi-head self-attention (QK^T -> softmax -> ctx -> linear) on 8 TRN2 cores.

Sharding: each core owns one (batch, query-block) shard: batch = core//4,
queries [qlo, qlo+512) with qlo = (core%4)*512. Attention needs all keys of
the core's batch, so keys are replicated per batch; no collectives needed.

All matmuls run in bf16 (fp32 PSUM accumulate). Per core (head h, its 512
queries q in 4 blocks of 128, all 2048 keys k in 16 blocks of 128):

  S_T[k, q]  = sum_d x[k, hd] * x[q, hd]          (PE; stationary xt, moving xq)
  P_T[k, q]  = exp(0.125 * S_T[k, q])             (ACT exp, or Pool pow for
                7 of 16 kb steps per pair: DVE stages the PSUM->SBUF copy and
                Pool computes (e^0.125)^s -- exp is the throughput wall, so
                it is spread over three engines)
  ctx[q, m]  = sum_k P_T[k, q] * xa[k, m]         (PE; stationary P_T 128q-slice,
                moving xa=[x|1|0] so col 64 is the softmax denominator; 66-wide
                output halves the PE column count vs the xa-stationary form)
  ctxn[q, i] = ctx[q, d] * (1/ctx[q, 64])         (DVE strided recip + per-
                partition tensor_scalar mul)
  ctxT       = DMA-transpose(ctxn)                (idle DMA xbar, no PE cost)
  out[q, o]  = sum_i ctxT[i, q] * W[o, i] + b[o]  (PE; bias via K=1 matmul)

Scheduling notes (TimelineSim cost model): Pool-offloaded steps give their
ctx matmuls 14 steps of pipeline slack (coarse sync, count-ordered with the
ACT stream) -- tighter couplings consistently lost ~10us to scheduler/cost-
model friction. ctx PSUM accumulation is bank-granular: one start=True
pending-zeroes a head's whole bank; the 4 query-block regions then rely on
byte-granular pending-zero, so accumulation order across kb is free.
Projection chunks 0..6 pre-accumulate into freed PSUM banks during the last
pair's normalization. ~148us vs 167us baseline; ACT ~98us, PE ~102us busy.
"""

import sys

for _p in ("/opt/trn_rl_repo", "/root/.axon_site/_ro/trn_rl_repo"):
    if _p not in sys.path:
        sys.path.append(_p)

import numpy as np
import ml_dtypes

import concourse.bacc as bacc
import concourse.mybir as mybir
import concourse.tile as tile

F32 = mybir.dt.float32
BF16 = mybir.dt.bfloat16

B, L, H, NH, DH = 2, 2048, 1024, 16, 64
NCORES = 8
QB = 512


def build_nc(L=2048, NH=16, DH=64, QB=512, H=1024):
    """One SPMD Bass program; per-core data differences live in the inputs."""
    KBLKS = L // 128            # key blocks of 128
    NPAIR = NH // 2             # head pairs (2 heads share a 128-part tile)
    AUG = DH + 2                # x + ones column + zero pad (ISA needs even width)
    HC = H // 128               # hidden chunks for the final matmul
    QSUB = QB // 128            # query sub-blocks of 128
    NSTEP = NPAIR * KBLKS
    SKEW = 6                    # ACT-region ctx matmuls trail scores by SKEW
    SKEWP = 13                  # Pool-region ctx matmuls trail further still
    # Every step's exp is split: ACT takes score columns [0:CA], Pool takes
    # [CA:2QB] via a DVE-staged PSUM->SBUF copy + pow. CA=640 aligns with the
    # 128-wide ctx stationary slices so each ctx matmul depends on only one
    # exp producer.
    CA = int(__import__('os').environ.get('CA', '1024'))
    # ctx_b's bank-zeroing start matmul lives in the ACT-region group, so the
    # ACT region must cover head-b qs0.
    assert CA >= QB + 128 or CA == 2 * QB
    BASE = float(np.exp(0.125))

    nc = bacc.Bacc("TRN2")
    xt = nc.declare_dram_parameter("xt", [NPAIR, 128, L], BF16, isOutput=False)
    xq = nc.declare_dram_parameter("xq", [NPAIR, 128, QB], BF16, isOutput=False)
    xa = nc.declare_dram_parameter("xa", [NH, 128, KBLKS * AUG], BF16, isOutput=False)
    wt = nc.declare_dram_parameter("wt", [128, HC * H], BF16, isOutput=False)
    biasrow = nc.declare_dram_parameter("biasrow", [1, H], BF16, isOutput=False)
    ones = nc.declare_dram_parameter("ones", [1, 128], BF16, isOutput=False)
    out = nc.declare_dram_parameter("out", [QB, H], F32, isOutput=True)

    with tile.TileContext(nc) as tc:
        with (
            tc.tile_pool(name="xt", bufs=3) as xt_pool,
            tc.tile_pool(name="xq", bufs=3) as xq_pool,
            tc.tile_pool(name="xa", bufs=6) as xa_pool,
            tc.tile_pool(name="p", bufs=11) as p_pool,
            tc.tile_pool(name="pp", bufs=10) as pp_pool,
            tc.tile_pool(name="scp", bufs=5) as scp_pool,
            tc.tile_pool(name="rc", bufs=4) as rc_pool,
            tc.tile_pool(name="ctxn", bufs=32) as ctxn_pool,
            tc.tile_pool(name="consts", bufs=1) as consts,
            tc.tile_pool(name="osb", bufs=2) as o_pool,
            tc.tile_pool(name="spsum", bufs=2, space="PSUM") as s_psum,
            tc.tile_pool(name="cpsum", bufs=4, space="PSUM") as c_psum,
        ):
            ones_t = consts.tile([1, 128], BF16)
            bias_t = consts.tile([1, H], BF16)
            base_t = consts.tile([128, 2 * QB], F32)
            nc.vector.memset(base_t[:], BASE)
            warm = consts.tile([128, 2], BF16)
            nc.scalar.activation(
                warm[:], base_t[:, 0:2], mybir.ActivationFunctionType.Exp
            )
            wt_t = consts.tile([128, HC * H], BF16)
            ctxT = [
                consts.tile([128, QB], BF16, tag=f"ctxT{c}", name=f"ctxT{c}")
                for c in range(HC)
            ]

            tiles = {}

            def emit_pair_dma(pr):
                xt_t = xt_pool.tile([128, L], BF16)
                xq_t = xq_pool.tile([128, QB], BF16)
                nc.sync.dma_start(xq_t[:], xq[pr])
                nc.sync.dma_start(xt_t[:, 0 : L // 4], xt[pr][:, 0 : L // 4])
                nc.sync.dma_start(xt_t[:, L // 4 : L // 2], xt[pr][:, L // 4 : L // 2])
                nc.sync.dma_start(xt_t[:, L // 2 :], xt[pr][:, L // 2 :])
                xa_a = xa_pool.tile([128, KBLKS * AUG], BF16)
                nc.sync.dma_start(xa_a[:], xa[2 * pr])
                xa_b = xa_pool.tile([128, KBLKS * AUG], BF16)
                nc.sync.dma_start(xa_b[:], xa[2 * pr + 1])
                # Full 2KB PSUM bank per head: start=True pending-zeroes the
                # whole bank, so the 4 qb accumulation regions share one
                # start (first matmul) and rely on byte-granular pending-zero.
                ctx_a = c_psum.tile([128, 512], F32, tag="c")
                ctx_b = c_psum.tile([128, 512], F32, tag="c")
                tiles[pr] = (xt_t, xq_t, xa_a, xa_b, ctx_a, ctx_b)

            def emit_norm_recips(pr):
                _, _, _, _, ctx_a, ctx_b = tiles[pr]
                rc_ab = rc_pool.tile([128, 2 * QSUB], F32)
                with nc.allow_low_precision(reason="softmax denom reciprocal"):
                    nc.vector.reciprocal(
                        rc_ab[:, 0:QSUB], ctx_a[:, DH : QSUB * AUG : AUG]
                    )
                    nc.vector.reciprocal(
                        rc_ab[:, QSUB : 2 * QSUB], ctx_b[:, DH : QSUB * AUG : AUG]
                    )
                return rc_ab

            ctxn_cur = [None]

            def emit_norm_mul(pr, qs, h, rc_ab):
                _, _, _, _, ctx_a, ctx_b = tiles[pr]
                asl = slice(qs * AUG, qs * AUG + DH)
                if h == 0:
                    ctxn_cur[0] = ctxn_pool.tile([128, 128], BF16, name="ctxn")
                    nc.vector.tensor_scalar(
                        ctxn_cur[0][:, 0:DH], ctx_a[:, asl],
                        rc_ab[:, qs : qs + 1], None,
                        op0=mybir.AluOpType.mult,
                    )
                else:
                    nc.vector.tensor_scalar(
                        ctxn_cur[0][:, DH:128], ctx_b[:, asl],
                        rc_ab[:, QSUB + qs : QSUB + qs + 1], None,
                        op0=mybir.AluOpType.mult,
                    )
                    nc.sync.dma_start(
                        ctxT[pr][:, qs * 128 : (qs + 1) * 128], ctxn_cur[0][:],
                        transpose=True,
                    )

            # Software-pipelined (pair, kb) stream. Steps kb in POOL_KB are
            # offloaded whole to the Pool engine (DVE stages the PSUM->SBUF
            # copy, Pool computes (e^0.125)^s); their ctx matmuls run at the
            # END of the same pair (kb 14/15), giving the slow pool chain
            # ~13 steps of slack with coarse-grained sync. The other steps
            # run ACT exp with ctx trailing by SKEW steps.
            POOL_KB = (0, 2, 4, 7, 9, 11, 13)
            emit_pair_dma(0)
            emit_pair_dma(1)
            pending = []
            pending_p = []
            norm_q = []
            pair_rc = {}
            act_ctx_done = set()

            ctx_cnt = {}

            def emit_ctx(prp, kbp, pp):
                cnt = ctx_cnt.get(prp, 0)
                ctx_cnt[prp] = cnt + 1
                first = cnt == 0
                last = cnt == KBLKS - 1
                _, _, xa_a, xa_b, ctx_a, ctx_b = tiles[prp]
                asl = slice(kbp * AUG, (kbp + 1) * AUG)
                for qs in range(QSUB):
                    qsl = slice(qs * 128, (qs + 1) * 128)
                    osl = slice(qs * AUG, (qs + 1) * AUG)
                    nc.tensor.matmul(
                        ctx_a[:, osl], pp[:, qsl], xa_a[:, asl],
                        start=(first and qs == 0), stop=last,
                        skip_group_check=True,
                    )
                for qs in range(QSUB):
                    qsl = slice(QB + qs * 128, QB + (qs + 1) * 128)
                    osl = slice(qs * AUG, (qs + 1) * AUG)
                    nc.tensor.matmul(
                        ctx_b[:, osl], pp[:, qsl], xa_b[:, asl],
                        start=(first and qs == 0), stop=last,
                        skip_group_check=True,
                    )
                if last:
                    norm_q.append(("r", prp))
                    for q_ in range(QSUB):
                        norm_q.append(("m", prp, q_))

            for gs in range(NSTEP + SKEW + 5 + KBLKS):
                if gs < NSTEP:
                    pr, kb = divmod(gs, KBLKS)
                    if kb == 0 and pr + 2 <= NPAIR - 1:
                        emit_pair_dma(pr + 2)
                    if pr == NPAIR - 2 and kb == 0:
                        nc.sync.dma_start(wt_t[:], wt[:])
                        nc.sync.dma_start(ones_t[:], ones[:])
                        nc.sync.dma_start(bias_t[:], biasrow[:])
                    xt_t, xq_t, xa_a, xa_b, _, _ = tiles[pr]
                    s_ab = s_psum.tile([128, 2 * QB], F32, tag="s")
                    ksl = slice(kb * 128, (kb + 1) * 128)
                    nc.tensor.matmul(
                        s_ab[:, 0:QB], xt_t[0:DH, ksl], xq_t[0:DH, :],
                        start=True, stop=True,
                    )
                    nc.tensor.matmul(
                        s_ab[:, QB : 2 * QB], xt_t[DH:128, ksl], xq_t[DH:128, :],
                        start=True, stop=True,
                    )
                    if kb in POOL_KB:
                        s_sb = scp_pool.tile([128, 2 * QB], F32)
                        nc.vector.tensor_copy(s_sb[:], s_ab[:])
                        p_pl = pp_pool.tile([128, 2 * QB], BF16, tag="pp")
                        nc.gpsimd.tensor_tensor(
                            p_pl[:], base_t[:], s_sb[:], op=mybir.AluOpType.pow
                        )
                        pending_p.append((pr, kb, p_pl))
                    else:
                        p_act = p_pool.tile([128, 2 * QB], BF16, tag="p")
                        nc.scalar.activation(
                            p_act[:], s_ab[:],
                            mybir.ActivationFunctionType.Exp, scale=0.125,
                        )
                        pending.append((pr, kb, p_act))
                else:
                    pr, kb = divmod(gs, KBLKS)
                    if not (pending or pending_p or norm_q):
                        break
                if len(pending) > SKEW or (gs >= NSTEP and pending):
                    prp, kbp, pp = pending.pop(0)
                    emit_ctx(prp, kbp, pp)
                if pending_p and (gs >= NSTEP or gs >= pending_p[0][0] * KBLKS + pending_p[0][1] + 14):
                    prp, kbp, pp = pending_p.pop(0)
                    emit_ctx(prp, kbp, pp)
                if norm_q and (gs >= NSTEP or (gs % KBLKS) in (6, 9, 10, 11, 12)):
                    op = norm_q.pop(0)
                    if op[0] == "r":
                        pair_rc[op[1]] = emit_norm_recips(op[1])
                    else:
                        emit_norm_mul(op[1], op[2], 0, pair_rc[op[1]])
                        emit_norm_mul(op[1], op[2], 1, pair_rc[op[1]])

            # Early projection: chunks 0..HC-2 for the first two query
            # blocks can accumulate while the last pair's norm finishes
            # (their ctxT chunks are complete; PSUM slots free as the last
            # exps retire).
            early_ps = {}
            for qs in (0, 1):
                qsl = slice(qs * 128, (qs + 1) * 128)
                out_ps = s_psum.tile([128, 2 * QB], F32, tag="s")
                early_ps[qs] = [out_ps[:, 0:QB], out_ps[:, QB : 2 * QB]]
                for c in range(HC - 1):
                    for ob in range(2):
                        nc.tensor.matmul(
                            early_ps[qs][ob], ctxT[c][:, qsl],
                            wt_t[:, c * H + ob * QB : c * H + (ob + 1) * QB],
                            start=(c == 0), stop=False,
                        )
            for qs in (2, 3):
                qsl = slice(qs * 128, (qs + 1) * 128)
                ps0 = c_psum.tile([128, 512], F32, tag="c")
                ps1 = c_psum.tile([128, 512], F32, tag="c")
                early_ps[qs] = [ps0[:, 0:QB], ps1[:, 0:QB]]
                for c in range(HC - 1):
                    for ob in range(2):
                        nc.tensor.matmul(
                            early_ps[qs][ob], ctxT[c][:, qsl],
                            wt_t[:, c * H + ob * QB : c * H + (ob + 1) * QB],
                            start=(c == 0), stop=False,
                        )
            for op in norm_q:
                if op[0] == "r":
                    pair_rc[op[1]] = emit_norm_recips(op[1])
                else:
                    emit_norm_mul(op[1], op[2], 0, pair_rc[op[1]])
                    emit_norm_mul(op[1], op[2], 1, pair_rc[op[1]])

            # Output projection: out[q, :] = sum_c ctxT_c[:, q].T @ wt_c + b
            # (moving operand is capped at 512 elements per matmul -> 2 blocks)
            c = HC - 1
            for qs in range(QSUB):
                qsl = slice(qs * 128, (qs + 1) * 128)
                for ob in range(2):
                    nc.tensor.matmul(
                        early_ps[qs][ob], ctxT[c][:, qsl],
                        wt_t[:, c * H + ob * QB : c * H + (ob + 1) * QB],
                        start=False, stop=False,
                    )
                    nc.tensor.matmul(
                        early_ps[qs][ob], ones_t[0:1, 0:128],
                        bias_t[0:1, ob * QB : (ob + 1) * QB],
                        start=False, stop=True,
                    )
            for qs in range(QSUB):
                o_sb = o_pool.tile([128, H], F32)
                for ob in range(2):
                    osl = slice(ob * QB, (ob + 1) * QB)
                    nc.vector.tensor_copy(o_sb[:, osl], early_ps[qs][ob])
                    nc.sync.dma_start(
                        out[qs * 128 : (qs + 1) * 128, ob * QB : (ob + 1) * QB],
                        o_sb[:, osl],
                    )
    nc.compile()
    return nc


def shard_inputs(key, W_ctx, b_ctx, L=2048, NH=16, DH=64, QB=512, H=1024):
    """Host-side prep of per-core input dicts (bf16)."""
    KBLKS = L // 128
    NPAIR = NH // 2
    AUG = DH + 2
    HC = H // 128
    Bv = key.shape[0]
    cores_per_batch = NCORES // Bv
    bf16 = ml_dtypes.bfloat16

    key = np.asarray(key, dtype=np.float32)
    xh = key.reshape(Bv, L, NH, DH)
    # xt: [B, NPAIR, 128, L], pair p rows 0:64 = head 2p, 64:128 = head 2p+1
    xt_full = np.ascontiguousarray(
        xh.transpose(0, 2, 3, 1).reshape(Bv, NPAIR, 2 * DH, L).astype(bf16)
    )
    # xa: [B, NH, 128, KBLKS*AUG] with ones in column kb*AUG+DH
    xa_full = np.empty((Bv, NH, 128, KBLKS * AUG), dtype=bf16)
    xa_view = xa_full.reshape(Bv, NH, 128, KBLKS, AUG)
    xa_view[..., DH] = 1.0
    xa_view[..., DH + 1] = 0.0
    xa_view[..., 0:DH] = (
        xh.reshape(Bv, KBLKS, 128, NH, DH).transpose(0, 3, 2, 1, 4).astype(bf16)
    )
    wt_host = np.ascontiguousarray(
        np.asarray(W_ctx, np.float32).T.reshape(HC, 128, H).transpose(1, 0, 2)
        .reshape(128, HC * H).astype(bf16)
    )
    bias_host = np.ascontiguousarray(
        np.asarray(b_ctx, np.float32).reshape(1, H).astype(bf16)
    )
    ones_host = np.ones((1, 128), dtype=bf16)

    in_maps = []
    meta = []
    for c in range(NCORES):
        b = c // cores_per_batch
        qlo = (c % cores_per_batch) * QB
        in_maps.append(
            {
                "xt": xt_full[b],
                "xq": np.ascontiguousarray(xt_full[b][:, :, qlo : qlo + QB]),
                "xa": xa_full[b],
                "wt": wt_host,
                "biasrow": bias_host,
                "ones": ones_host,
            }
        )
        meta.append((b, qlo))
    return in_maps, meta


_NC_CACHE = {}


def kernel(key, W_ctx, b_ctx):
    from concourse.bass_utils import run_bass_kernel_spmd

    key = np.asarray(key, dtype=np.float32)
    if "nc" not in _NC_CACHE:
        _NC_CACHE["nc"] = build_nc(L=L, NH=NH, DH=DH, QB=QB, H=H)
    nc = _NC_CACHE["nc"]
    in_maps, meta = shard_inputs(key, W_ctx, b_ctx, L=L, NH=NH, DH=DH, QB=QB, H=H)
    res = run_bass_kernel_spmd(nc, in_maps, list(range(NCORES)))
    outf = np.empty((B, L, H), dtype=np.float32)
    for c, (b, qlo) in enumerate(meta):
        outf[b, qlo : qlo + QB] = res.results[c]["out"]
    return outf
